# revision 21
# baseline (speedup 1.0000x reference)
"""Trainium2 Bass kernel for nn_CrossAttention_G (sparse_attention).

Math (per batch b):
    k      = y @ W_qk.T
    patch  = softmax((x @ k.T) * D**-0.5, axis=-1)          # content scores
    pos    = softmax(coords @ pos_emb, axis=-1)             # positional, batch-shared
    g      = sigmoid(gating)
    attn   = (1-g)*patch + g*pos
    out    = attn @ y
    ent    = sum(-attn*log(attn+1e-8), -1)
    hmap   = 2*(1 - sigmoid(temp*ent)) = 2*sigmoid(temp*H), H = -ent

Sharding: data-parallel over B=16 across 8 cores (2 batches/core);
coords/W_qk/pos_emb replicated. All heavy compute on-device; host only
shards/concats and reshapes.

Device algorithm per core (fp32 storage, float32r PE matmuls):
  pos:    s_pos[p,n] = sum_k coords[p,n,k]*pos_emb[p,k] done on PE as 6
          accumulating matmuls with diag(pos_emb[:,k]) stationary and a
          stride-6 view of coords as moving operand; softmax fused via
          ACT Exp(accum_out=Z); gpos = g*pos stored bf16.
  per b:  x,y loaded; xT,yT built via PE transposes; qT = W.T-tiles @ xT;
          scores(nt) = qT.T @ yT in PSUM; softmax via reduce_max + ACT
          Exp(scale=SCALE, bias=-SCALE*max, accum_out=Z); blend with gpos;
          entropy via tensor_tensor_reduce; attn transposed on PE; out =
          attnT.T @ y; H -> sigmoid -> hmap on device.
"""

import math

import numpy as np

B, N, D = 16, 1024, 768
NCORES = 8
BPC = B // NCORES  # batches per core
SCALE = D ** -0.5
P = 128
NT = N // P  # 8 row tiles
DT = D // P  # 6 feature tiles
KC = 6       # coords inner dim
CQ = 4       # coords quarters per p-tile
CQN = N // CQ  # 256 n per quarter

_CACHE: dict = {}


def _build(g: float, temp: float, stage: int = 99):
    import concourse.bass as bass
    import concourse.mybir as mybir
    from concourse import bacc
    import concourse.tile as tile
    from concourse.masks import make_identity

    fp32 = mybir.dt.float32
    f32r = mybir.dt.float32r
    bf16 = mybir.dt.bfloat16
    AF = mybir.ActivationFunctionType
    OP = mybir.AluOpType
    AX = mybir.AxisListType

    nc = bacc.Bacc("TRN2", target_bir_lowering=False, debug=False)

    xs = nc.dram_tensor("xs", [BPC, N, D], f32r, kind="ExternalInput").ap()
    ys = nc.dram_tensor("ys", [BPC, N, D], f32r, kind="ExternalInput").ap()
    coords = nc.dram_tensor("coords", [N, N, KC], f32r, kind="ExternalInput").ap()
    wqk = nc.dram_tensor("wqk", [D, D], f32r, kind="ExternalInput").ap()
    pemb = nc.dram_tensor("pemb", [N, KC], fp32, kind="ExternalInput").ap()
    out_d = nc.dram_tensor("out", [BPC, N, D], fp32, kind="ExternalOutput").ap()
    hmap_d = nc.dram_tensor("hmap", [BPC, P, NT], fp32, kind="ExternalOutput").ap()

    with tile.TileContext(nc) as tc:
        with (
            tc.tile_pool(name="const", bufs=1) as constp,
            tc.tile_pool(name="gpos", bufs=NT) as gposp,
            tc.tile_pool(name="coords", bufs=2) as coordp,
            tc.tile_pool(name="diag", bufs=2) as diagp,
            tc.tile_pool(name="ynat", bufs=1) as ynatp,
            tc.tile_pool(name="xnat", bufs=2) as xnatp,
            tc.tile_pool(name="bigT", bufs=1) as bigp,
            tc.tile_pool(name="tmp", bufs=3) as tmpp,
            tc.tile_pool(name="attn", bufs=2) as attnp,
            tc.tile_pool(name="attnT", bufs=2) as attnTp,
            tc.tile_pool(name="outsb", bufs=2) as outp,
            tc.tile_pool(name="rows", bufs=24) as rowp,
            tc.tile_pool(name="hrow", bufs=2 * BPC) as hrowp,
            tc.tile_pool(name="psA", bufs=2, space="PSUM") as psA,   # [128,1024] x2 = 4 banks
            tc.tile_pool(name="psT", bufs=2, space="PSUM") as psT,   # [128,512]  x2 = 2 banks
            tc.tile_pool(name="psO", bufs=1, space="PSUM") as psO,   # [128,768]  x1 = 2 banks
        ):
            ident_f = constp.tile([P, P], fp32, tag="identf")
            make_identity(nc, ident_f[:])
            ident = constp.tile([P, P], f32r, tag="ident")
            nc.vector.tensor_copy(ident[:], ident_f[:])

            eps_row = constp.tile([P, 1], fp32, tag="eps")
            nc.gpsimd.memset(eps_row[:], 1e-8)

            pe_sb = constp.tile([P, NT * KC], fp32, tag="pemb")
            nc.sync.dma_start(out=pe_sb[:].rearrange("p (t k) -> p t k", k=KC),
                              in_=pemb.rearrange("(t p) k -> p t k", p=P))

            w_sb = constp.tile([P, DT * D], f32r, tag="w")
            nc.sync.dma_start(out=w_sb[:].rearrange("p (t d) -> p t d", d=D),
                              in_=wqk.rearrange("(t p) d -> p t d", p=P))

            # ---------------- positional scores (batch-shared) ----------------
            gpos_tiles = []
            for pt in range(NT if stage >= 1 else 0):
                dgf = diagp.tile([P, KC * P], fp32, tag="diagf")
                for k in range(KC):
                    # diag(pos_emb[pt*128:(pt+1)*128, k]) on the idle GpSimd engine
                    nc.gpsimd.tensor_scalar_mul(
                        dgf[:, k * P:(k + 1) * P], ident_f[:],
                        pe_sb[:, pt * KC + k:pt * KC + k + 1],
                    )
                dg = diagp.tile([P, KC * P], f32r, tag="diag")
                nc.vector.tensor_copy(dg[:], dgf[:])
                ps_pos = psA.tile([P, N], fp32, tag="psA")
                for q in range(CQ):
                    ct = coordp.tile([P, CQN * KC], f32r, tag="coords")
                    nc.sync.dma_start(
                        out=ct[:],
                        in_=coords[pt * P:(pt + 1) * P, q * CQN:(q + 1) * CQN, :]
                        .rearrange("p n k -> p (n k)"),
                    )
                    cv = ct[:].rearrange("p (n k) -> p n k", k=KC)
                    for k in range(KC):
                        nc.tensor.matmul(
                            ps_pos[:, q * CQN:(q + 1) * CQN],
                            dg[:, k * P:(k + 1) * P],
                            cv[:, :, k],
                            start=(k == 0), stop=(k == KC - 1),
                        )
                mx = rowp.tile([P, 1], fp32, tag="mx")
                nc.vector.reduce_max(mx[:], ps_pos[:], axis=AX.X)
                nb = rowp.tile([P, 1], fp32, tag="nb")
                nc.vector.tensor_scalar_mul(nb[:], mx[:], -1.0)
                pexp = tmpp.tile([P, N], fp32, tag="tmp")
                zz = rowp.tile([P, 1], fp32, tag="zz")
                nc.scalar.activation(pexp[:], ps_pos[:], AF.Exp, bias=nb[:],
                                     scale=1.0, accum_out=zz[:])
                rz = rowp.tile([P, 1], fp32, tag="rz")
                nc.vector.reciprocal(rz[:], zz[:])
                rzg = rowp.tile([P, 1], fp32, tag="rzg")
                nc.vector.tensor_scalar_mul(rzg[:], rz[:], float(g))
                gp = gposp.tile([P, N], bf16, tag="gpos")
                nc.vector.tensor_scalar_mul(gp[:], pexp[:], rzg[:])
                gpos_tiles.append(gp)

            # ---------------- per-batch attention ----------------
            h_tiles = []
            for b in range(BPC if stage >= 2 else 0):
                ynat = ynatp.tile([P, NT * D], f32r, tag="ynat")
                nc.sync.dma_start(out=ynat[:].rearrange("p (t d) -> p t d", d=D),
                                  in_=ys[b].rearrange("(t p) d -> p t d", p=P))

                xT = bigp.tile([P, DT * N], f32r, tag="xT")
                yT = bigp.tile([P, DT * N], f32r, tag="yT")
                qT = bigp.tile([P, DT * N], f32r, tag="qT")
                xTv = xT[:].rearrange("p (t n) -> p t n", t=DT)
                yTv = yT[:].rearrange("p (t n) -> p t n", t=DT)
                qTv = qT[:].rearrange("p (t n) -> p t n", t=DT)

                # transpose x and y into [d, n] layout via PE
                for src, dstv, is_x in ((None, xTv, True), (ynat, yTv, False)):
                    for nt in range(NT):
                        if is_x:
                            xn = xnatp.tile([P, D], f32r, tag="xnat")
                            nc.sync.dma_start(out=xn[:],
                                              in_=xs[b, nt * P:(nt + 1) * P, :])
                            s_ap = xn[:]
                        else:
                            s_ap = src[:, nt * D:(nt + 1) * D]
                        pt1 = psT.tile([P, 4 * P], f32r, tag="psT")
                        for dt in range(4):
                            nc.tensor.transpose(pt1[:, dt * P:(dt + 1) * P],
                                                s_ap[:, dt * P:(dt + 1) * P],
                                                ident[:])
                        nc.vector.tensor_copy(
                            dstv[:, 0:4, nt * P:(nt + 1) * P],
                            pt1[:].rearrange("p (t n) -> p t n", t=4))
                        pt2 = psT.tile([P, 4 * P], f32r, tag="psT")
                        for dt in range(4, DT):
                            nc.tensor.transpose(pt2[:, (dt - 4) * P:(dt - 3) * P],
                                                s_ap[:, dt * P:(dt + 1) * P],
                                                ident[:])
                        nc.vector.tensor_copy(
                            dstv[:, 4:DT, nt * P:(nt + 1) * P],
                            pt2[:, 0:(DT - 4) * P].rearrange("p (t n) -> p t n", t=DT - 4))

                # qT[d, n] = sum_e W[e, d] * xT[e, n]
                for dt in range(DT if stage >= 3 else 0):
                    ps_q = psA.tile([P, N], fp32, tag="psA")
                    for ch in range(2):
                        for et in range(DT):
                            nc.tensor.matmul(
                                ps_q[:, ch * 512:(ch + 1) * 512],
                                w_sb[:, et * D + dt * P: et * D + (dt + 1) * P],
                                xTv[:, et, ch * 512:(ch + 1) * 512],
                                start=(et == 0), stop=(et == DT - 1),
                            )
                    nc.scalar.copy(qTv[:, dt, :], ps_q[:])

                h_sb = hrowp.tile([P, NT], fp32, tag="h")
                h_tiles.append(h_sb)

                for nt in range(NT if stage >= 4 else 0):
                    ps_s = psA.tile([P, N], fp32, tag="psA")
                    for ch in range(2):
                        for dt in range(DT):
                            nc.tensor.matmul(
                                ps_s[:, ch * 512:(ch + 1) * 512],
                                qTv[:, dt, nt * P:(nt + 1) * P],
                                yTv[:, dt, ch * 512:(ch + 1) * 512],
                                start=(dt == 0), stop=(dt == DT - 1),
                            )
                    mx = rowp.tile([P, 1], fp32, tag="mx")
                    nc.vector.reduce_max(mx[:], ps_s[:], axis=AX.X)
                    nb = rowp.tile([P, 1], fp32, tag="nb")
                    nc.vector.tensor_scalar_mul(nb[:], mx[:], -SCALE)
                    esb = tmpp.tile([P, N], fp32, tag="tmp")
                    zz = rowp.tile([P, 1], fp32, tag="zz")
                    nc.scalar.activation(esb[:], ps_s[:], AF.Exp, bias=nb[:],
                                         scale=SCALE, accum_out=zz[:])
                    rz = rowp.tile([P, 1], fp32, tag="rz")
                    nc.vector.reciprocal(rz[:], zz[:])
                    rza = rowp.tile([P, 1], fp32, tag="rzg")
                    nc.vector.tensor_scalar_mul(rza[:], rz[:], float(1.0 - g))
                    attn = attnp.tile([P, N], fp32, tag="attn")
                    nc.vector.tensor_scalar_mul(attn[:], esb[:], rza[:])
                    nc.vector.tensor_add(attn[:], attn[:], gpos_tiles[nt][:])
                    # entropy: H = sum(attn * log(attn + 1e-8))
                    lat = tmpp.tile([P, N], fp32, tag="tmp")
                    nc.scalar.activation(lat[:], attn[:], AF.Ln, bias=eps_row[:], scale=1.0)
                    # NOTE: tensor_tensor_reduce crashes this runtime (custom
                    # DVE ucode unavailable); use mul + reduce instead.
                    ttro = tmpp.tile([P, N], fp32, tag="tmp")
                    nc.vector.tensor_mul(ttro[:], attn[:], lat[:])
                    nc.vector.reduce_sum(h_sb[:, nt:nt + 1], ttro[:], axis=AX.X)
                    # attn^T via PE, then out[nt] = sum_mt attnT.T @ y
                    if stage < 5:
                        continue
                    aT = attnTp.tile([P, N], f32r, tag="attnT")
                    for gblk in range(2):
                        pt_ = psT.tile([P, 4 * P], fp32, tag="psT")
                        for j in range(4):
                            mt = gblk * 4 + j
                            nc.tensor.transpose(pt_[:, j * P:(j + 1) * P],
                                                attn[:, mt * P:(mt + 1) * P],
                                                ident_f[:])
                        nc.scalar.copy(aT[:, gblk * 512:(gblk + 1) * 512], pt_[:])
                    ps_o = psO.tile([P, D], fp32, tag="psO")
                    for co, cw in ((0, 512), (512, 256)):
                        for mt in range(NT):
                            nc.tensor.matmul(
                                ps_o[:, co:co + cw],
                                aT[:, mt * P:(mt + 1) * P],
                                ynat[:, mt * D + co: mt * D + co + cw],
                                start=(mt == 0), stop=(mt == NT - 1),
                            )
                    osb = outp.tile([P, D], fp32, tag="outsb")
                    nc.scalar.copy(osb[:], ps_o[:])
                    nc.sync.dma_start(out=out_d[b, nt * P:(nt + 1) * P, :], in_=osb[:])

            # hmap = 2*sigmoid(temp*H) = 2/(1+exp(-temp*H)); use Exp set (no
            # sigmoid table-set switch)
            for b in range(BPC if stage >= 6 else 0):
                hm = hrowp.tile([P, NT], fp32, tag="hm")
                nc.scalar.activation(hm[:], h_tiles[b][:], AF.Exp,
                                     bias=0.0, scale=float(-temp))
                if stage == 62:  # exp only
                    nc.sync.dma_start(out=hmap_d[b], in_=hm[:])
                    continue
                hm1 = hrowp.tile([P, NT], fp32, tag="hm1")
                nc.vector.tensor_scalar_add(hm1[:], hm[:], 1.0)
                hmr = hrowp.tile([P, NT], fp32, tag="hmr")
                nc.vector.reciprocal(hmr[:], hm1[:])
                if stage == 63:  # through reciprocal
                    nc.sync.dma_start(out=hmap_d[b], in_=hmr[:])
                    continue
                hm2 = hrowp.tile([P, NT], fp32, tag="hm2")
                nc.vector.tensor_scalar_mul(hm2[:], hmr[:], 2.0)
                nc.sync.dma_start(out=hmap_d[b], in_=hm2[:])

    nc.compile()
    return nc


def kernel(**inputs):
    x = np.ascontiguousarray(np.asarray(inputs["x"], dtype=np.float32))
    y = np.ascontiguousarray(np.asarray(inputs["y"], dtype=np.float32))
    coords = np.ascontiguousarray(np.asarray(inputs["coords"], dtype=np.float32))
    w = np.ascontiguousarray(np.asarray(inputs["W_qk"], dtype=np.float32))
    pemb = np.ascontiguousarray(
        np.asarray(inputs["pos_emb"], dtype=np.float32).reshape(N, KC))
    gating = float(np.asarray(inputs["gating"], dtype=np.float32))
    temp = float(np.asarray(inputs["temp"], dtype=np.float32))
    g = 1.0 / (1.0 + math.exp(-gating))

    key = (round(g, 12), round(temp, 12))
    if key not in _CACHE:
        _CACHE[key] = _build(g, temp)
    nc = _CACHE[key]

    from concourse.bass_utils import run_bass_kernel_spmd

    in_maps = []
    for c in range(NCORES):
        sl = slice(c * BPC, (c + 1) * BPC)
        in_maps.append({
            "xs": x[sl], "ys": y[sl], "coords": coords, "wqk": w, "pemb": pemb,
        })
    res = run_bass_kernel_spmd(nc, in_maps, core_ids=list(range(NCORES)))
    outs = res.results
    out = np.concatenate([np.asarray(r["out"]) for r in outs], axis=0)
    hmap = np.concatenate([np.asarray(r["hmap"]) for r in outs], axis=0)
    # device stores hmap as [b, p, t]; row n = t*128 + p
    hmap = np.transpose(hmap, (0, 2, 1)).reshape(B, N, 1)
    return out.reshape(B, N, D), hmap


# revision 22
# speedup vs baseline: 1.1805x; 1.1805x over previous
"""Trainium2 Bass kernel for nn_CrossAttention_G (sparse_attention).

Math (per batch b):
    k      = y @ W_qk.T
    patch  = softmax((x @ k.T) * D**-0.5, axis=-1)          # content scores
    pos    = softmax(coords @ pos_emb, axis=-1)             # positional, batch-shared
    g      = sigmoid(gating)
    attn   = (1-g)*patch + g*pos
    out    = attn @ y
    ent    = sum(-attn*log(attn+1e-8), -1)
    hmap   = 2*(1 - sigmoid(temp*ent)) = 2*sigmoid(temp*H), H = -ent

Sharding: data-parallel over B=16 across 8 cores (2 batches/core);
coords/W_qk/pos_emb replicated. All heavy compute on-device; host only
shards/concats and reshapes.

Device algorithm per core (fp32 storage, float32r PE matmuls):
  pos:    s_pos[p,n] = sum_k coords[p,n,k]*pos_emb[p,k] done on PE as 6
          accumulating matmuls with diag(pos_emb[:,k]) stationary and a
          stride-6 view of coords as moving operand; softmax fused via
          ACT Exp(accum_out=Z); gpos = g*pos stored bf16.
  per b:  x,y loaded; xT,yT built via PE transposes; qT = W.T-tiles @ xT;
          scores(nt) = qT.T @ yT in PSUM; softmax via reduce_max + ACT
          Exp(scale=SCALE, bias=-SCALE*max, accum_out=Z); blend with gpos;
          entropy via tensor_tensor_reduce; attn transposed on PE; out =
          attnT.T @ y; H -> sigmoid -> hmap on device.
"""

import math

import numpy as np

B, N, D = 16, 1024, 768
NCORES = 8
BPC = B // NCORES  # batches per core
SCALE = D ** -0.5
P = 128
NT = N // P  # 8 row tiles
DT = D // P  # 6 feature tiles
KC = 6       # coords inner dim
CQ = 4       # coords quarters per p-tile
CQN = N // CQ  # 256 n per quarter

_CACHE: dict = {}


def _build(g: float, temp: float, stage: int = 99):
    import concourse.bass as bass
    import concourse.mybir as mybir
    from concourse import bacc
    import concourse.tile as tile
    from concourse.masks import make_identity

    fp32 = mybir.dt.float32
    f32r = mybir.dt.float32r
    bf16 = mybir.dt.bfloat16
    AF = mybir.ActivationFunctionType
    OP = mybir.AluOpType
    AX = mybir.AxisListType

    from concourse import hw_specs

    if not getattr(hw_specs, "_ant_act_patch", False):
        _orig_gat = hw_specs.get_activation_tables

        def _gat(module_arch):
            t = _orig_gat(module_arch)
            if "natural_log_exp_and_others" in t:
                # leave ids (dict order) untouched; just remove exp/ln from
                # the competing sets so both activations pick the shared set
                for name, funcs in t.items():
                    if name != "natural_log_exp_and_others":
                        funcs.discard(mybir.ActivationFunctionType.Exp)
                        funcs.discard(mybir.ActivationFunctionType.Ln)
            return t

        hw_specs.get_activation_tables = _gat
        hw_specs._ant_act_patch = True
        for _m in (bacc,):
            if hasattr(_m, "get_activation_tables"):
                _m.get_activation_tables = _gat

    nc = bacc.Bacc("TRN2", target_bir_lowering=False, debug=False)

    xs = nc.dram_tensor("xs", [BPC, N, D], f32r, kind="ExternalInput").ap()
    ys = nc.dram_tensor("ys", [BPC, N, D], f32r, kind="ExternalInput").ap()
    coords = nc.dram_tensor("coords", [N, N, KC], f32r, kind="ExternalInput").ap()
    wqk = nc.dram_tensor("wqk", [D, D], f32r, kind="ExternalInput").ap()
    pemb = nc.dram_tensor("pemb", [N, KC], fp32, kind="ExternalInput").ap()
    out_d = nc.dram_tensor("out", [BPC, N, D], fp32, kind="ExternalOutput").ap()
    hmap_d = nc.dram_tensor("hmap", [BPC, P, NT], fp32, kind="ExternalOutput").ap()

    with tile.TileContext(nc) as tc:
        with (
            tc.tile_pool(name="const", bufs=1) as constp,
            tc.tile_pool(name="gpos", bufs=NT) as gposp,
            tc.tile_pool(name="coords", bufs=2) as coordp,
            tc.tile_pool(name="diag", bufs=2) as diagp,
            tc.tile_pool(name="ynat", bufs=1) as ynatp,
            tc.tile_pool(name="xnat", bufs=2) as xnatp,
            tc.tile_pool(name="bigT", bufs=1) as bigp,
            tc.tile_pool(name="tmp", bufs=3) as tmpp,
            tc.tile_pool(name="attn", bufs=2) as attnp,
            tc.tile_pool(name="attnT", bufs=2) as attnTp,
            tc.tile_pool(name="outsb", bufs=2) as outp,
            tc.tile_pool(name="rows", bufs=24) as rowp,
            tc.tile_pool(name="hrow", bufs=2 * BPC) as hrowp,
            tc.tile_pool(name="psA", bufs=2, space="PSUM") as psA,   # [128,1024] x2 = 4 banks
            tc.tile_pool(name="psT", bufs=2, space="PSUM") as psT,   # [128,512]  x2 = 2 banks
            tc.tile_pool(name="psO", bufs=1, space="PSUM") as psO,   # [128,768]  x1 = 2 banks
        ):
            ident_f = constp.tile([P, P], fp32, tag="identf")
            make_identity(nc, ident_f[:])
            ident = constp.tile([P, P], f32r, tag="ident")
            nc.vector.tensor_copy(ident[:], ident_f[:])

            eps_row = constp.tile([P, 1], fp32, tag="eps")
            nc.gpsimd.memset(eps_row[:], 1e-8)

            pe_sb = constp.tile([P, NT * KC], fp32, tag="pemb")
            nc.sync.dma_start(out=pe_sb[:].rearrange("p (t k) -> p t k", k=KC),
                              in_=pemb.rearrange("(t p) k -> p t k", p=P))

            w_sb = constp.tile([P, DT * D], f32r, tag="w")
            nc.sync.dma_start(out=w_sb[:].rearrange("p (t d) -> p t d", d=D),
                              in_=wqk.rearrange("(t p) d -> p t d", p=P))

            # ---------------- positional scores (batch-shared) ----------------
            gpos_tiles = []
            for pt in range(NT if stage >= 1 else 0):
                dg = diagp.tile([P, KC * P], f32r, tag="diag")
                for k in range(KC):
                    # diag(pos_emb[pt*128:(pt+1)*128, k]) on DVE (GpSimd is 4x
                    # slower here and serialized the whole pos phase)
                    nc.vector.tensor_scalar_mul(
                        dg[:, k * P:(k + 1) * P], ident_f[:],
                        pe_sb[:, pt * KC + k:pt * KC + k + 1],
                    )
                ps_pos = psA.tile([P, N], fp32, tag="psA")
                for q in range(CQ):
                    ct = coordp.tile([P, CQN * KC], f32r, tag="coords")
                    nc.sync.dma_start(
                        out=ct[:],
                        in_=coords[pt * P:(pt + 1) * P, q * CQN:(q + 1) * CQN, :]
                        .rearrange("p n k -> p (n k)"),
                    )
                    cv = ct[:].rearrange("p (n k) -> p n k", k=KC)
                    for k in range(KC):
                        nc.tensor.matmul(
                            ps_pos[:, q * CQN:(q + 1) * CQN],
                            dg[:, k * P:(k + 1) * P],
                            cv[:, :, k],
                            start=(k == 0), stop=(k == KC - 1),
                        )
                mx = rowp.tile([P, 1], fp32, tag="mx")
                nc.vector.reduce_max(mx[:], ps_pos[:], axis=AX.X)
                nb = rowp.tile([P, 1], fp32, tag="nb")
                nc.vector.tensor_scalar_mul(nb[:], mx[:], -1.0)
                pexp = tmpp.tile([P, N], fp32, tag="tmp")
                zz = rowp.tile([P, 1], fp32, tag="zz")
                nc.scalar.activation(pexp[:], ps_pos[:], AF.Exp, bias=nb[:],
                                     scale=1.0, accum_out=zz[:])
                rz = rowp.tile([P, 1], fp32, tag="rz")
                nc.vector.reciprocal(rz[:], zz[:])
                rzg = rowp.tile([P, 1], fp32, tag="rzg")
                nc.vector.tensor_scalar_mul(rzg[:], rz[:], float(g))
                gp = gposp.tile([P, N], bf16, tag="gpos")
                nc.vector.tensor_scalar_mul(gp[:], pexp[:], rzg[:])
                gpos_tiles.append(gp)

            # ---------------- per-batch attention ----------------
            h_tiles = []
            for b in range(BPC if stage >= 2 else 0):
                ynat = ynatp.tile([P, NT * D], f32r, tag="ynat")
                nc.sync.dma_start(out=ynat[:].rearrange("p (t d) -> p t d", d=D),
                                  in_=ys[b].rearrange("(t p) d -> p t d", p=P))

                xT = bigp.tile([P, DT * N], f32r, tag="xT")
                yT = bigp.tile([P, DT * N], f32r, tag="yT")
                qT = bigp.tile([P, DT * N], f32r, tag="qT")
                xTv = xT[:].rearrange("p (t n) -> p t n", t=DT)
                yTv = yT[:].rearrange("p (t n) -> p t n", t=DT)
                qTv = qT[:].rearrange("p (t n) -> p t n", t=DT)

                # transpose x and y into [d, n] layout via PE
                for src, dstv, is_x in ((None, xTv, True), (ynat, yTv, False)):
                    for nt in range(NT):
                        if is_x:
                            xn = xnatp.tile([P, D], f32r, tag="xnat")
                            nc.sync.dma_start(out=xn[:],
                                              in_=xs[b, nt * P:(nt + 1) * P, :])
                            s_ap = xn[:]
                        else:
                            s_ap = src[:, nt * D:(nt + 1) * D]
                        pt1 = psT.tile([P, 4 * P], f32r, tag="psT")
                        for dt in range(4):
                            nc.tensor.transpose(pt1[:, dt * P:(dt + 1) * P],
                                                s_ap[:, dt * P:(dt + 1) * P],
                                                ident[:])
                        nc.vector.tensor_copy(
                            dstv[:, 0:4, nt * P:(nt + 1) * P],
                            pt1[:].rearrange("p (t n) -> p t n", t=4))
                        pt2 = psT.tile([P, 4 * P], f32r, tag="psT")
                        for dt in range(4, DT):
                            nc.tensor.transpose(pt2[:, (dt - 4) * P:(dt - 3) * P],
                                                s_ap[:, dt * P:(dt + 1) * P],
                                                ident[:])
                        nc.vector.tensor_copy(
                            dstv[:, 4:DT, nt * P:(nt + 1) * P],
                            pt2[:, 0:(DT - 4) * P].rearrange("p (t n) -> p t n", t=DT - 4))

                # qT[d, n] = sum_e W[e, d] * xT[e, n]
                for dt in range(DT if stage >= 3 else 0):
                    ps_q = psA.tile([P, N], fp32, tag="psA")
                    for ch in range(2):
                        for et in range(DT):
                            nc.tensor.matmul(
                                ps_q[:, ch * 512:(ch + 1) * 512],
                                w_sb[:, et * D + dt * P: et * D + (dt + 1) * P],
                                xTv[:, et, ch * 512:(ch + 1) * 512],
                                start=(et == 0), stop=(et == DT - 1),
                            )
                    nc.scalar.copy(qTv[:, dt, :], ps_q[:])

                h_sb = hrowp.tile([P, NT], fp32, tag="h")
                h_tiles.append(h_sb)

                for nt in range(NT if stage >= 4 else 0):
                    ps_s = psA.tile([P, N], fp32, tag="psA")
                    for ch in range(2):
                        for dt in range(DT):
                            nc.tensor.matmul(
                                ps_s[:, ch * 512:(ch + 1) * 512],
                                qTv[:, dt, nt * P:(nt + 1) * P],
                                yTv[:, dt, ch * 512:(ch + 1) * 512],
                                start=(dt == 0), stop=(dt == DT - 1),
                            )
                    mx = rowp.tile([P, 1], fp32, tag="mx")
                    nc.vector.reduce_max(mx[:], ps_s[:], axis=AX.X)
                    nb = rowp.tile([P, 1], fp32, tag="nb")
                    nc.vector.tensor_scalar_mul(nb[:], mx[:], -SCALE)
                    esb = tmpp.tile([P, N], fp32, tag="tmp")
                    zz = rowp.tile([P, 1], fp32, tag="zz")
                    nc.scalar.activation(esb[:], ps_s[:], AF.Exp, bias=nb[:],
                                         scale=SCALE, accum_out=zz[:])
                    rz = rowp.tile([P, 1], fp32, tag="rz")
                    nc.vector.reciprocal(rz[:], zz[:])
                    rza = rowp.tile([P, 1], fp32, tag="rzg")
                    nc.vector.tensor_scalar_mul(rza[:], rz[:], float(1.0 - g))
                    attn = attnp.tile([P, N], fp32, tag="attn")
                    nc.vector.tensor_scalar_mul(attn[:], esb[:], rza[:])
                    nc.vector.tensor_add(attn[:], attn[:], gpos_tiles[nt][:])
                    # entropy: H = sum(attn * log(attn + 1e-8))
                    lat = tmpp.tile([P, N], fp32, tag="tmp")
                    nc.scalar.activation(lat[:], attn[:], AF.Ln, bias=eps_row[:], scale=1.0)
                    # NOTE: tensor_tensor_reduce crashes this runtime (custom
                    # DVE ucode unavailable); use mul + reduce instead.
                    ttro = tmpp.tile([P, N], fp32, tag="tmp")
                    nc.vector.tensor_mul(ttro[:], attn[:], lat[:])
                    nc.vector.reduce_sum(h_sb[:, nt:nt + 1], ttro[:], axis=AX.X)
                    # attn^T via PE, then out[nt] = sum_mt attnT.T @ y
                    if stage < 5:
                        continue
                    aT = attnTp.tile([P, N], f32r, tag="attnT")
                    for gblk in range(2):
                        pt_ = psT.tile([P, 4 * P], fp32, tag="psT")
                        for j in range(4):
                            mt = gblk * 4 + j
                            nc.tensor.transpose(pt_[:, j * P:(j + 1) * P],
                                                attn[:, mt * P:(mt + 1) * P],
                                                ident_f[:])
                        nc.scalar.copy(aT[:, gblk * 512:(gblk + 1) * 512], pt_[:])
                    ps_o = psO.tile([P, D], fp32, tag="psO")
                    for co, cw in ((0, 512), (512, 256)):
                        for mt in range(NT):
                            nc.tensor.matmul(
                                ps_o[:, co:co + cw],
                                aT[:, mt * P:(mt + 1) * P],
                                ynat[:, mt * D + co: mt * D + co + cw],
                                start=(mt == 0), stop=(mt == NT - 1),
                            )
                    osb = outp.tile([P, D], fp32, tag="outsb")
                    nc.scalar.copy(osb[:], ps_o[:])
                    nc.sync.dma_start(out=out_d[b, nt * P:(nt + 1) * P, :], in_=osb[:])

            # hmap = 2*sigmoid(temp*H) = 2/(1+exp(-temp*H)); use Exp set (no
            # sigmoid table-set switch)
            for b in range(BPC if stage >= 6 else 0):
                hm = hrowp.tile([P, NT], fp32, tag="hm")
                nc.scalar.activation(hm[:], h_tiles[b][:], AF.Exp,
                                     bias=0.0, scale=float(-temp))
                if stage == 62:  # exp only
                    nc.sync.dma_start(out=hmap_d[b], in_=hm[:])
                    continue
                hm1 = hrowp.tile([P, NT], fp32, tag="hm1")
                nc.vector.tensor_scalar_add(hm1[:], hm[:], 1.0)
                hmr = hrowp.tile([P, NT], fp32, tag="hmr")
                nc.vector.reciprocal(hmr[:], hm1[:])
                if stage == 63:  # through reciprocal
                    nc.sync.dma_start(out=hmap_d[b], in_=hmr[:])
                    continue
                hm2 = hrowp.tile([P, NT], fp32, tag="hm2")
                nc.vector.tensor_scalar_mul(hm2[:], hmr[:], 2.0)
                nc.sync.dma_start(out=hmap_d[b], in_=hm2[:])

    nc.compile()
    return nc


def kernel(**inputs):
    x = np.ascontiguousarray(np.asarray(inputs["x"], dtype=np.float32))
    y = np.ascontiguousarray(np.asarray(inputs["y"], dtype=np.float32))
    coords = np.ascontiguousarray(np.asarray(inputs["coords"], dtype=np.float32))
    w = np.ascontiguousarray(np.asarray(inputs["W_qk"], dtype=np.float32))
    pemb = np.ascontiguousarray(
        np.asarray(inputs["pos_emb"], dtype=np.float32).reshape(N, KC))
    gating = float(np.asarray(inputs["gating"], dtype=np.float32))
    temp = float(np.asarray(inputs["temp"], dtype=np.float32))
    g = 1.0 / (1.0 + math.exp(-gating))

    key = (round(g, 12), round(temp, 12))
    if key not in _CACHE:
        _CACHE[key] = _build(g, temp)
    nc = _CACHE[key]

    from concourse.bass_utils import run_bass_kernel_spmd

    in_maps = []
    for c in range(NCORES):
        sl = slice(c * BPC, (c + 1) * BPC)
        in_maps.append({
            "xs": x[sl], "ys": y[sl], "coords": coords, "wqk": w, "pemb": pemb,
        })
    res = run_bass_kernel_spmd(nc, in_maps, core_ids=list(range(NCORES)))
    outs = res.results
    out = np.concatenate([np.asarray(r["out"]) for r in outs], axis=0)
    hmap = np.concatenate([np.asarray(r["hmap"]) for r in outs], axis=0)
    # device stores hmap as [b, p, t]; row n = t*128 + p
    hmap = np.transpose(hmap, (0, 2, 1)).reshape(B, N, 1)
    return out.reshape(B, N, D), hmap
